# revision 38
# baseline (speedup 1.0000x reference)
"""Multi-head attention (B=2, S=2048, D=1024, H=16, dk=dv=64) on 8 TRN2 NeuronCores.

Sharding: core c -> (batch b = c//4, head-group g = c%4, 4 heads each).
Each core computes q/k/v projections for its 4 heads (weight-column shard),
attention over its batch, and a partial output projection over its 256
channels (weight-row shard of Wo).  The host sums the 4 partial outputs per
batch at unshard time (the "all-reduce after the output projection").

Host-side shard prep:
  * X slices are transposed to [D, S] so the contraction dim (D) lands on
    SBUF partitions for the projection matmuls.
  * The key-padding mask is applied by COMPACTION: masked keys are removed
    (gathered) from K/V before they ever reach the device.
  * The softmax 1/sqrt(dk) scale is folded into Wq/bq.
  * All matmul operands are bf16 (same PE cycles/col as fp32r, far lower
    power -> less DVFS throttling, half the HBM traffic and LDWEIGHTS time).
    fp8 was measured offline to blow the 2e-2 error budget (scores reach
    17.5 so exp overflows e5m2; e4m3 q/k gives 8.9e-2).

Scheduling: the softmax exp stream on the ACT engine (64 x [128,1024] tiles
~= 71us) is the attention-phase floor, so emission is ordered to start it as
early as possible and never starve it:
  * DMA order xk | xv | xq: the V projection (64 small matmuls) executes
    while the xq stream is still landing, instead of serializing after it.
  * PSUM: 2x[128,1024] score tiles + 1x[128,1024] ctx accumulator +
    2x[128,512] "filler" tiles (8 banks total).  The filler pool runs the
    late Q-projection (m=1) and all output-projection chunks *inside* the
    attention j-loops without perturbing the score-tile rotation.
  * All PSUM evacuations run on the DVE (ACT bf16 casts measured 3x slower),
    and no input DMA rides the ACT engine's DGE queue (DMA posts occupy the
    issuing engine's instruction stream).
  * The softmax normalize is DMA-free: a rank-1 PE matmul (ones x den_row)
    replicates the denominator across 64 partitions in PSUM, then a fast
    custom-DVE reciprocal + multiply normalize the context.  Each head's
    chain is deferred into the next head's filler slots so it never blocks
    score matmuls.
  * A short block of dependency-free warmup matmuls holds the PE's DVFS/ramp
    state up while the first inputs land (removing it measured +25us).
The output bias bo is added on the host during the partial-sum gather.
"""
import ml_dtypes
import numpy as np

B, S, D = 2, 2048, 1024
H, DK, DV = 16, 64, 64
SCALE = float(np.sqrt(DK))
NCORES = 8
GROUPS = 4           # head-groups (cores per batch)
HPG = H // GROUPS    # heads per core = 4
CH = HPG * DK        # channels per core = 256
MC = CH // 128       # c-chunks = 2
DJ = D // 128        # contraction chunks = 8
NQC = S // 128       # 16
P = 128

_BUILD_CACHE = {}
LAST_RESULTS = None  # test harness can read exec_time_ns etc. from here


def _bf16(a: np.ndarray) -> np.ndarray:
    return np.ascontiguousarray(a, dtype=np.float32).astype(ml_dtypes.bfloat16)


def _build(n_kp: int):
    """Build + schedule the per-core Bass program for a padded key count."""
    import concourse.bass as bass  # noqa: F401
    from concourse import bacc, tile, mybir

    DT = mybir.dt
    F32, BF16 = DT.float32, DT.bfloat16
    AF = mybir.ActivationFunctionType
    ALU = mybir.AluOpType

    NJ = n_kp // P                      # k-chunks
    NKB = (n_kp + 511) // 512           # 512-wide k blocks for the k projection

    nc = bacc.Bacc("TRN2", target_bir_lowering=False, debug=False,
                   num_devices=NCORES)

    xqT = nc.dram_tensor("xqT", [D, S], BF16, kind="ExternalInput")
    xkT = nc.dram_tensor("xkT", [D, n_kp], BF16, kind="ExternalInput")
    xvT = nc.dram_tensor("xvT", [D, n_kp], BF16, kind="ExternalInput")
    wqT = nc.dram_tensor("wqT", [D, CH], BF16, kind="ExternalInput")
    wkT = nc.dram_tensor("wkT", [D, CH], BF16, kind="ExternalInput")
    wvT = nc.dram_tensor("wvT", [D, CH], BF16, kind="ExternalInput")
    woT = nc.dram_tensor("woT", [CH, D], BF16, kind="ExternalInput")
    bq = nc.dram_tensor("bq", [CH], F32, kind="ExternalInput")
    bk = nc.dram_tensor("bk", [CH], F32, kind="ExternalInput")
    bv = nc.dram_tensor("bv", [CH], F32, kind="ExternalInput")
    valid = nc.dram_tensor("valid", [n_kp], F32, kind="ExternalInput")
    out = nc.dram_tensor("out", [S, D], BF16, kind="ExternalOutput")

    with tile.TileContext(nc) as tc:
        with (
            tc.tile_pool(name="xs", bufs=16) as xs,
            tc.tile_pool(name="persist", bufs=1) as pp,
            tc.tile_pool(name="exps", bufs=14) as ep,
            tc.tile_pool(name="scratch", bufs=3) as scr,
            tc.tile_pool(name="outs", bufs=3) as op,
            tc.tile_pool(name="cu", bufs=3) as cu,
            tc.tile_pool(name="psw", bufs=2, space="PSUM") as psw,
            tc.tile_pool(name="psc", bufs=1, space="PSUM") as psc,
            tc.tile_pool(name="pfill", bufs=2, space="PSUM") as pfill,
        ):
            # ---- persistent tiles -----------------------------------------
            wq_sb = pp.tile([P, DJ, CH], BF16, name="wq_sb")
            wk_sb = pp.tile([P, DJ, CH], BF16, name="wk_sb")
            wv_sb = pp.tile([P, DJ, CH], BF16, name="wv_sb")
            wo_sb = pp.tile([P, MC, D], BF16, name="wo_sb")
            bq_sb = pp.tile([P, MC], F32, name="bq_sb")
            bk_sb = pp.tile([P, MC], F32, name="bk_sb")
            qT_sb = pp.tile([P, MC, S], BF16, name="qT_sb")
            kT_sb = pp.tile([P, MC, n_kp], BF16, name="kT_sb")
            vaug = pp.tile([P, NJ, HPG, DV + 1], BF16, name="vaug")
            ctxN = pp.tile([P, MC, S], BF16, name="ctxN")

            # ---- DMA stream (issue order == consumption order) ------------
            # Bulk inputs ride ONLY the two hardware DGE queues (sync +
            # scalar); gpsimd DMA is software-driven by the Pool engine
            # itself and drags.  Order: xk | xv | wq | xq in q-column blocks
            # | wo, so K-proj, V-proj and Q-proj qb0/qb1 (all that head 0
            # needs) are ready as early as the aggregate ~300 GB/s allows.
            nc.sync.dma_start(out=wk_sb[:, 0, :], in_=wkT.ap()[0:P, :])
            nc.sync.dma_start(out=bk_sb[:], in_=bk.ap().rearrange("(m p) -> p m", p=P))
            xk_t = [xs.tile([P, S], BF16, tag="x", name=f"xk{dj}") for dj in range(DJ)]
            for dj in range(1, DJ):
                nc.sync.dma_start(out=wk_sb[:, dj, :], in_=wkT.ap()[dj * P:(dj + 1) * P, :])
            for dj in range(DJ):
                eng = nc.sync if dj % 2 == 0 else nc.gpsimd
                eng.dma_start(out=xk_t[dj][:, :n_kp], in_=xkT.ap()[dj * P:(dj + 1) * P, :])
            nc.sync.dma_start(out=bq_sb[:], in_=bq.ap().rearrange("(m p) -> p m", p=P))
            bv_rep = pp.tile([P, CH], F32, name="bv_rep")
            nc.gpsimd.dma_start(out=bv_rep[:], in_=bv.ap()[None, :].partition_broadcast(P))
            valid_sb = pp.tile([P, NJ], F32, name="valid_sb")
            nc.sync.dma_start(out=valid_sb[:], in_=valid.ap().rearrange("(j p) -> p j", p=P))
            valid_bf = pp.tile([P, NJ], BF16, name="valid_bf")
            nc.vector.tensor_copy(out=valid_bf[:], in_=valid_sb[:])

            for dj in range(DJ):
                nc.sync.dma_start(out=wq_sb[:, dj, :], in_=wqT.ap()[dj * P:(dj + 1) * P, :])
            xq_t = [xs.tile([P, S], BF16, tag="x", name=f"xq{dj}") for dj in range(DJ)]
            xv_t = [xs.tile([P, S], BF16, tag="x", name=f"xv{dj}") for dj in range(DJ)]
            qcnt = 0

            def xq_block(qb):
                nonlocal_marker = 0  # noqa
                for dj in range(DJ):
                    eng = nc.sync if dj % 4 != 3 else nc.gpsimd
                    eng.dma_start(
                        out=xq_t[dj][:, qb * 512:(qb + 1) * 512],
                        in_=xqT.ap()[dj * P:(dj + 1) * P, qb * 512:(qb + 1) * 512])

            xq_block(0)
            xq_block(1)
            for dj in range(DJ):
                nc.gpsimd.dma_start(out=wv_sb[:, dj, :], in_=wvT.ap()[dj * P:(dj + 1) * P, :])
            for dj in range(DJ):
                eng = nc.sync if dj % 2 == 0 else nc.gpsimd
                eng.dma_start(out=xv_t[dj][:, :n_kp], in_=xvT.ap()[dj * P:(dj + 1) * P, :])
            xq_block(2)
            xq_block(3)
            for m2 in range(MC):
                nc.sync.dma_start(out=wo_sb[:, m2, :], in_=woT.ap()[m2 * P:(m2 + 1) * P, :])

            # ---- constants + PE warmup ------------------------------------
            F32R = DT.float32r
            ones_f = pp.tile([P, DV], F32, name="ones_f")
            nc.vector.memset(ones_f[:, :], 1.0)
            ones_t = pp.tile([P, DV], F32R, name="ones_t")
            nc.vector.tensor_copy(out=ones_t[:, :], in_=ones_f[:, :])
            warm = pp.tile([P, 512], BF16, name="warm")
            nc.vector.memset(warm[:, :], 0.0)
            wout = scr.tile([P, 512], mybir.dt.float32, tag="s", name="wout")

            def warmup(n):
                # dependency-free matmuls that keep the PE's DVFS/ramp state
                # high while the input DMA stream is still landing (removing
                # these measured +25us: the ramp matters more than the duty
                # budget they consume)
                wps = pfill.tile([P, 512], mybir.dt.float32, tag="pf")
                for i in range(n):
                    nc.tensor.matmul(wps[:, :512], lhsT=warm[:, 0:P],
                                     rhs=warm[:, :512],
                                     start=(i == 0), stop=(i == n - 1))
                nc.vector.tensor_copy(out=wout[:], in_=wps[:, :512])
            # ---- projection group emitters (each -> one 1-bank pfill tile) -
            def kgroup(m, kb):
                w = min(512, n_kp - kb * 512)
                ps = pfill.tile([P, 512], mybir.dt.float32, tag="pf",
                                name=f"kg{m}{kb}")
                for dj in range(DJ):
                    nc.tensor.matmul(
                        ps[:, :w],
                        lhsT=wk_sb[:, dj, m * P:(m + 1) * P],
                        rhs=xk_t[dj][:, kb * 512:kb * 512 + w],
                        start=(dj == 0), stop=(dj == DJ - 1))
                nc.vector.tensor_scalar(
                    out=kT_sb[:, m, kb * 512:kb * 512 + w], in0=ps[:, :w],
                    scalar1=bk_sb[:, m:m + 1], scalar2=None, op0=ALU.add)

            def qgroup(m, qb):
                ps = pfill.tile([P, 512], mybir.dt.float32, tag="pf",
                                name=f"qg{m}{qb}")
                for dj in range(DJ):
                    nc.tensor.matmul(
                        ps[:, :512],
                        lhsT=wq_sb[:, dj, m * P:(m + 1) * P],
                        rhs=xq_t[dj][:, qb * 512:(qb + 1) * 512],
                        start=(dj == 0), stop=(dj == DJ - 1))
                nc.vector.tensor_scalar(
                    out=qT_sb[:, m, qb * 512:(qb + 1) * 512], in0=ps[:, :512],
                    scalar1=bq_sb[:, m:m + 1], scalar2=None, op0=ALU.add)

            def qgroup_gen(m, qb):
                ps = pfill.tile([P, 512], mybir.dt.float32, tag="pf",
                                name=f"qg{m}{qb}")
                for dj in range(DJ):
                    nc.tensor.matmul(
                        ps[:, :512],
                        lhsT=wq_sb[:, dj, m * P:(m + 1) * P],
                        rhs=xq_t[dj][:, qb * 512:(qb + 1) * 512],
                        start=(dj == 0), stop=(dj == DJ - 1))
                    if dj in (2, 5):
                        yield
                nc.vector.tensor_scalar(
                    out=qT_sb[:, m, qb * 512:(qb + 1) * 512], in0=ps[:, :512],
                    scalar1=bq_sb[:, m:m + 1], scalar2=None, op0=ALU.add)

            def vgroup(j):
                ps = pfill.tile([P, 512], mybir.dt.float32, tag="pf",
                                name=f"vg{j}")
                for dj in range(DJ):
                    nc.tensor.matmul(
                        ps[:, :CH],
                        lhsT=xv_t[dj][:, j * P:(j + 1) * P],
                        rhs=wv_sb[:, dj, :],
                        start=(dj == 0), stop=(dj == DJ - 1))
                vst = scr.tile([P, 1024], mybir.dt.float32, tag="s")
                nc.vector.tensor_tensor(out=vst[:, :CH], in0=ps[:, :CH],
                                        in1=bv_rep[:], op=ALU.add)
                nc.vector.tensor_scalar(
                    out=vaug[:, j, :, 0:DV],
                    in0=vst[:, :CH].rearrange("p (h d) -> p h d", h=HPG),
                    scalar1=valid_sb[:, j:j + 1], scalar2=None, op0=ALU.mult)
                for h in range(HPG):
                    nc.gpsimd.tensor_copy(out=vaug[:, j, h, DV:DV + 1],
                                          in_=valid_bf[:, j:j + 1])

            _stages = {}

            def opgroup(qc, n2):
                ps = pfill.tile([P, 512], mybir.dt.float32, tag="pf",
                                name=f"og{qc}{n2}")
                for m in range(MC):
                    nc.tensor.matmul(
                        ps[:, :512],
                        lhsT=ctxN[:, m, qc * P:(qc + 1) * P],
                        rhs=wo_sb[:, m, n2 * 512:(n2 + 1) * 512],
                        start=(m == 0), stop=(m == MC - 1))
                if n2 == 0:
                    _stages[qc] = op.tile([P, D], BF16, tag="o", name=f"os{qc}")
                stage = _stages[qc]
                nc.vector.tensor_copy(out=stage[:, n2 * 512:(n2 + 1) * 512],
                                      in_=ps[:, :512])
                if n2 == 1:
                    nc.sync.dma_start(out=out.ap()[qc * P:(qc + 1) * P, :],
                                      in_=stage[:])
                    del _stages[qc]

            # ---- attention: one flat software-pipelined stream --------------
            # All 8 (q-half, head) score loops run as a single continuous
            # iteration stream on the PE.  AV (attn @ V) matmuls trail the
            # score/exp stream through a pending queue that drains up to two
            # per iteration, so head boundaries cost no PE or ACT bubbles.
            # Head (0,0) runs scores-only: its AV backlog (and the whole V
            # projection, which is still waiting on the xv DMA) drains inside
            # head (0,1)'s iterations.
            ctx_of = {}
            pend = []           # (half, h, j, ex) awaiting their AV matmuls
            fillq = []          # filler generators/thunks; [0] may be started
            fill_started = [False]
            v_emitted = [0]     # vgroups emitted so far (gates h(0,0) AVs)

            import types as _types

            def fill_slot():
                # emit one filler chunk: generators resume (a few matmuls per
                # slot) so a single 8-matmul projection group never blocks the
                # exp stream for 3us straight.
                while fillq:
                    f = fillq[0]
                    if not isinstance(f, _types.GeneratorType):
                        fillq.pop(0)
                        f()
                        return True
                    try:
                        next(f)
                        fill_started[0] = True
                        return True
                    except StopIteration:
                        fillq.pop(0)
                        fill_started[0] = False
                return False

            def push_priority(f):
                # normalize-finish closures must precede any not-yet-started
                # static filler (out-proj readers of ctxN rely on program
                # order), but must not split an in-flight generator.
                idx = 1 if (fillq and fill_started[0]) else 0
                fillq.insert(idx, f)

            def emit_av(half, h, j, ex):
                key = (half, h)
                if key not in ctx_of:
                    ctx_of[key] = psc.tile([P, 1024], mybir.dt.float32,
                                           tag="ctx", name=f"ctx{half}{h}")
                ctx_ps = ctx_of[key]
                for qq in range(2):
                    nc.tensor.matmul(
                        ctx_ps[0:DV + 1, qq * 512:(qq + 1) * 512],
                        lhsT=vaug[:, j, h, :],
                        rhs=ex[:, qq * 512:(qq + 1) * 512],
                        start=(j == 0), stop=(j == NJ - 1))
                if j == NJ - 1:
                    finish_head(half, h)

            def finish_head(half, h):
                q0 = half * 1024
                m, po = h // 2, (h % 2) * 64
                ctx_ps = ctx_of.pop((half, h))
                ctxU = cu.tile([P, 1024], F32R, tag="cu", name=f"cu{half}{h}")
                nc.vector.tensor_copy(out=ctxU[0:DV + 1, :],
                                      in_=ctx_ps[0:DV + 1, :])

                def fin():
                    # rank-1 PE broadcast of the denominator row, fast
                    # custom-DVE reciprocal from PSUM, then normalize.
                    rec = scr.tile([P, 1024], mybir.dt.float32, tag="s",
                                   name=f"rc{half}{h}")
                    for qq in range(2):
                        dps = pfill.tile([P, 512], mybir.dt.float32, tag="pf",
                                         name=f"dn{half}{h}{qq}")
                        nc.tensor.matmul(
                            dps[0:DV, :512], lhsT=ones_t[64:65, :],
                            rhs=ctxU[64:65, qq * 512:(qq + 1) * 512],
                            start=True, stop=True)
                        nc.vector.reciprocal_approx_fast(
                            out=rec[0:DV, qq * 512:(qq + 1) * 512],
                            in_=dps[0:DV, :512])
                    if po == 0:
                        nc.vector.tensor_tensor(out=ctxN[0:64, m, q0:q0 + 1024],
                                                in0=ctxU[0:64, :],
                                                in1=rec[0:64, :], op=ALU.mult)
                    else:
                        tmp = scr.tile([P, 1024], BF16, tag="s",
                                       name=f"tm{half}{h}")
                        nc.vector.tensor_tensor(out=tmp[0:64, :],
                                                in0=ctxU[0:64, :],
                                                in1=rec[0:64, :], op=ALU.mult)
                        nc.gpsimd.dma_start(out=ctxN[64:128, m, q0:q0 + 1024],
                                            in_=tmp[0:64, :])
                push_priority(fin)

            def drain_av(limit, keep=1):
                n = 0
                while n < limit and len(pend) > keep:
                    half, h, j, ex = pend[0]
                    if (half, h) == (0, 0) and j >= v_emitted[0]:
                        break       # vaug[j] not emitted yet
                    pend.pop(0)
                    emit_av(half, h, j, ex)
                    n += 1

            def flat_head(half, h, fillers=(), av=True):
                for f in fillers:
                    fillq.append(f)
                q0 = half * 1024
                m, po = h // 2, (h % 2) * 64
                for j in range(NJ):
                    st = psw.tile([P, 1024], mybir.dt.float32, tag="ps",
                                  name=f"st{half}{h}{j}")
                    for qq in range(2):
                        nc.tensor.matmul(
                            st[:, qq * 512:(qq + 1) * 512],
                            lhsT=kT_sb[po:po + 64, m, j * P:(j + 1) * P],
                            rhs=qT_sb[po:po + 64, m,
                                      q0 + qq * 512:q0 + (qq + 1) * 512],
                            start=True, stop=True)
                    ex = ep.tile([P, 1024], BF16, tag="e", name=f"ex{half}{h}{j}")
                    nc.scalar.activation(out=ex[:], in_=st[:], func=AF.Exp)
                    pend.append((half, h, j, ex))
                    if av:
                        drain_av(2)
                    fill_slot()
                # drain this head's leftover fillers before moving on
                while fillq:
                    fill_slot()

            warmup(24)
            for kb in range(NKB):
                kgroup(0, kb)
            warmup(6)
            for kb in range(NKB):
                kgroup(1, kb)
            warmup(12)
            qgroup(0, 0)
            qgroup(0, 1)
            flat_head(0, 0, av=False)
            for j in range(NJ):
                vgroup(j)
                v_emitted[0] = j + 1
                drain_av(1)
            flat_head(0, 1, fillers=[qgroup_gen(1, 0), qgroup_gen(1, 1)])
            flat_head(0, 2, fillers=[qgroup_gen(0, 2), qgroup_gen(0, 3)])
            flat_head(0, 3, fillers=[qgroup_gen(1, 2), qgroup_gen(1, 3)])
            flat_head(1, 1, fillers=[lambda: opgroup(0, 0), lambda: opgroup(0, 1)])
            flat_head(1, 3, fillers=[
                lambda: opgroup(1, 0), lambda: opgroup(1, 1),
                lambda: opgroup(2, 0), lambda: opgroup(2, 1)])
            flat_head(1, 0, fillers=[
                lambda: opgroup(3, 0), lambda: opgroup(3, 1),
                lambda: opgroup(4, 0), lambda: opgroup(4, 1)])
            flat_head(1, 2, fillers=[
                lambda: opgroup(5, 0), lambda: opgroup(5, 1),
                lambda: opgroup(6, 0), lambda: opgroup(6, 1),
                lambda: opgroup(7, 0), lambda: opgroup(7, 1)])
            drain_av(len(pend), keep=0)
            while fillq:
                fill_slot()

            # tail out-projection on fat [128,1024] tiles (attention PSUM is
            # free by now): half the instruction count of the 512-wide form.
            for qc in range(8, NQC):
                ps = psw.tile([P, 1024], mybir.dt.float32, tag="ps",
                              name=f"tops{qc}")
                for n2 in range(2):
                    for m in range(MC):
                        nc.tensor.matmul(
                            ps[:, n2 * 512:(n2 + 1) * 512],
                            lhsT=ctxN[:, m, qc * P:(qc + 1) * P],
                            rhs=wo_sb[:, m, n2 * 512:(n2 + 1) * 512],
                            start=(m == 0), stop=(m == MC - 1))
                stage = op.tile([P, D], BF16, tag="o", name=f"tos{qc}")
                nc.vector.tensor_copy(out=stage[:], in_=ps[:])
                eng = (nc.sync, nc.scalar, nc.gpsimd)[qc % 3]
                eng.dma_start(out=out.ap()[qc * P:(qc + 1) * P, :], in_=stage[:])

    nc.compile()
    return nc


def _ensure_axon_hooks():
    """bass_utils imports antenv.axon_hooks when tracing; this image's antenv
    lacks it. Provide it, backed by the ctypes NTFF hook when available."""
    import sys
    import types
    try:
        import antenv.axon_hooks  # noqa: F401
        return
    except ImportError:
        pass
    hook = None
    try:
        from trn_agent_boot.trn_boot import _ntff_profile_via_ctypes
        hook = _ntff_profile_via_ctypes("/opt/axon/libaxon_pjrt.so")
    except Exception:
        hook = None
    mod = types.ModuleType("antenv.axon_hooks")
    mod._hook = hook
    mod.get_axon_ntff_profile_hook = lambda: mod._hook
    mod.set_axon_ntff_profile_hook = lambda h: setattr(mod, "_hook", h)
    sys.modules["antenv.axon_hooks"] = mod


def kernel(Q, K, V, atte_mask_out, Wq, bq, Wk, bk, Wv, bv, Wo, bo):
    import jax  # noqa: F401  (must be imported first so the axon backend registers)
    from concourse.bass_utils import run_bass_kernel_spmd
    global LAST_RESULTS
    _ensure_axon_hooks()

    Q = np.asarray(Q); K = np.asarray(K); V = np.asarray(V)
    mask = np.asarray(atte_mask_out).reshape(B, S)
    Wq = np.asarray(Wq); Wk = np.asarray(Wk); Wv = np.asarray(Wv); Wo = np.asarray(Wo)
    bq = np.asarray(bq); bk = np.asarray(bk); bv = np.asarray(bv); bo = np.asarray(bo)

    keep = [np.flatnonzero(~mask[b]) for b in range(B)]
    n_kp = max(P, max(((len(ix) + P - 1) // P) * P for ix in keep))

    # per-batch packed (and bf16-rounded) tensors
    xqT, xkT, xvT, validv = [], [], [], []
    for b in range(B):
        ix = keep[b]
        xqT.append(_bf16(Q[b].T))
        kk = np.zeros((D, n_kp), np.float32)
        vv = np.zeros((D, n_kp), np.float32)
        kk[:, :len(ix)] = K[b][ix].T
        vv[:, :len(ix)] = V[b][ix].T
        xkT.append(_bf16(kk))
        xvT.append(_bf16(vv))
        va = np.zeros(n_kp, np.float32)
        va[:len(ix)] = 1.0
        validv.append(va)

    in_maps = []
    for c in range(NCORES):
        b, g = c // GROUPS, c % GROUPS
        sl = slice(g * CH, (g + 1) * CH)
        in_maps.append({
            "xqT": xqT[b], "xkT": xkT[b], "xvT": xvT[b],
            "wqT": _bf16(Wq[sl].T / SCALE),
            "wkT": _bf16(Wk[sl].T),
            "wvT": _bf16(Wv[sl].T),
            "woT": _bf16(Wo[:, sl].T),
            "bq": np.ascontiguousarray(bq[sl] / SCALE, np.float32),
            "bk": np.ascontiguousarray(bk[sl], np.float32),
            "bv": np.ascontiguousarray(bv[sl], np.float32),
            "valid": validv[b],
        })

    if n_kp not in _BUILD_CACHE:
        _BUILD_CACHE[n_kp] = _build(n_kp)
    nc = _BUILD_CACHE[n_kp]

    res = run_bass_kernel_spmd(nc, in_maps, core_ids=list(range(NCORES)))
    LAST_RESULTS = res

    full = np.zeros((B, S, D), np.float32)
    full += bo.astype(np.float32)
    for c in range(NCORES):
        full[c // GROUPS] += np.asarray(res.results[c]["out"], np.float32)
    return full


# revision 39
# speedup vs baseline: 1.0068x; 1.0068x over previous
"""Multi-head attention (B=2, S=2048, D=1024, H=16, dk=dv=64) on 8 TRN2 NeuronCores.

Sharding: core c -> (batch b = c//4, head-group g = c%4, 4 heads each).
Each core computes q/k/v projections for its 4 heads (weight-column shard),
attention over its batch, and a partial output projection over its 256
channels (weight-row shard of Wo).  The host sums the 4 partial outputs per
batch at unshard time (the "all-reduce after the output projection").

Host-side shard prep:
  * X slices are transposed to [D, S] so the contraction dim (D) lands on
    SBUF partitions for the projection matmuls.
  * The key-padding mask is applied by COMPACTION: masked keys are removed
    (gathered) from K/V before they ever reach the device.
  * The softmax 1/sqrt(dk) scale is folded into Wq/bq.
  * All matmul operands are bf16 (same PE cycles/col as fp32r, far lower
    power -> less DVFS throttling, half the HBM traffic and LDWEIGHTS time).
    fp8 was measured offline to blow the 2e-2 error budget (scores reach
    17.5 so exp overflows e5m2; e4m3 q/k gives 8.9e-2).

Scheduling: the softmax exp stream on the ACT engine (64 x [128,1024] tiles
~= 71us) is the attention-phase floor, so emission is ordered to start it as
early as possible and never starve it:
  * DMA order xk | xv | xq: the V projection (64 small matmuls) executes
    while the xq stream is still landing, instead of serializing after it.
  * PSUM: 2x[128,1024] score tiles + 1x[128,1024] ctx accumulator +
    2x[128,512] "filler" tiles (8 banks total).  The filler pool runs the
    late Q-projection (m=1) and all output-projection chunks *inside* the
    attention j-loops without perturbing the score-tile rotation.
  * All PSUM evacuations run on the DVE (ACT bf16 casts measured 3x slower),
    and no input DMA rides the ACT engine's DGE queue (DMA posts occupy the
    issuing engine's instruction stream).
  * The softmax normalize is DMA-free: a rank-1 PE matmul (ones x den_row)
    replicates the denominator across 64 partitions in PSUM, then a fast
    custom-DVE reciprocal + multiply normalize the context.  Each head's
    chain is deferred into the next head's filler slots so it never blocks
    score matmuls.
  * A short block of dependency-free warmup matmuls holds the PE's DVFS/ramp
    state up while the first inputs land (removing it measured +25us).
The output bias bo is added on the host during the partial-sum gather.
"""
import ml_dtypes
import numpy as np

B, S, D = 2, 2048, 1024
H, DK, DV = 16, 64, 64
SCALE = float(np.sqrt(DK))
NCORES = 8
GROUPS = 4           # head-groups (cores per batch)
HPG = H // GROUPS    # heads per core = 4
CH = HPG * DK        # channels per core = 256
MC = CH // 128       # c-chunks = 2
DJ = D // 128        # contraction chunks = 8
NQC = S // 128       # 16
P = 128

_BUILD_CACHE = {}
LAST_RESULTS = None  # test harness can read exec_time_ns etc. from here


def _bf16(a: np.ndarray) -> np.ndarray:
    return np.ascontiguousarray(a, dtype=np.float32).astype(ml_dtypes.bfloat16)


def _build(n_kp: int):
    """Build + schedule the per-core Bass program for a padded key count."""
    import concourse.bass as bass  # noqa: F401
    from concourse import bacc, tile, mybir

    DT = mybir.dt
    F32, BF16 = DT.float32, DT.bfloat16
    AF = mybir.ActivationFunctionType
    ALU = mybir.AluOpType

    NJ = n_kp // P                      # k-chunks
    NKB = (n_kp + 511) // 512           # 512-wide k blocks for the k projection

    nc = bacc.Bacc("TRN2", target_bir_lowering=False, debug=False,
                   num_devices=NCORES)

    xqT = nc.dram_tensor("xqT", [D, S], BF16, kind="ExternalInput")
    xkT = nc.dram_tensor("xkT", [D, n_kp], BF16, kind="ExternalInput")
    xvT = nc.dram_tensor("xvT", [D, n_kp], BF16, kind="ExternalInput")
    wqT = nc.dram_tensor("wqT", [D, CH], BF16, kind="ExternalInput")
    wkT = nc.dram_tensor("wkT", [D, CH], BF16, kind="ExternalInput")
    wvT = nc.dram_tensor("wvT", [D, CH], BF16, kind="ExternalInput")
    woT = nc.dram_tensor("woT", [CH, D], BF16, kind="ExternalInput")
    bq = nc.dram_tensor("bq", [CH], F32, kind="ExternalInput")
    bk = nc.dram_tensor("bk", [CH], F32, kind="ExternalInput")
    bv = nc.dram_tensor("bv", [CH], F32, kind="ExternalInput")
    valid = nc.dram_tensor("valid", [n_kp], F32, kind="ExternalInput")
    out = nc.dram_tensor("out", [S, D], BF16, kind="ExternalOutput")

    with tile.TileContext(nc) as tc:
        with (
            tc.tile_pool(name="xs", bufs=16) as xs,
            tc.tile_pool(name="persist", bufs=1) as pp,
            tc.tile_pool(name="exps", bufs=14) as ep,
            tc.tile_pool(name="scratch", bufs=3) as scr,
            tc.tile_pool(name="outs", bufs=3) as op,
            tc.tile_pool(name="cu", bufs=3) as cu,
            tc.tile_pool(name="psw", bufs=2, space="PSUM") as psw,
            tc.tile_pool(name="psc", bufs=1, space="PSUM") as psc,
            tc.tile_pool(name="pfill", bufs=2, space="PSUM") as pfill,
        ):
            # ---- persistent tiles -----------------------------------------
            wq_sb = pp.tile([P, DJ, CH], BF16, name="wq_sb")
            wk_sb = pp.tile([P, DJ, CH], BF16, name="wk_sb")
            wv_sb = pp.tile([P, DJ, CH], BF16, name="wv_sb")
            wo_sb = pp.tile([P, MC, D], BF16, name="wo_sb")
            bq_sb = pp.tile([P, MC], F32, name="bq_sb")
            bk_sb = pp.tile([P, MC], F32, name="bk_sb")
            qT_sb = pp.tile([P, MC, S], BF16, name="qT_sb")
            kT_sb = pp.tile([P, MC, n_kp], BF16, name="kT_sb")
            vaug = pp.tile([P, NJ, HPG, DV + 1], BF16, name="vaug")
            ctxN = pp.tile([P, MC, S], BF16, name="ctxN")

            # ---- DMA stream (issue order == consumption order) ------------
            # Bulk inputs ride ONLY the two hardware DGE queues (sync +
            # scalar); gpsimd DMA is software-driven by the Pool engine
            # itself and drags.  Order: xk | xv | wq | xq in q-column blocks
            # | wo, so K-proj, V-proj and Q-proj qb0/qb1 (all that head 0
            # needs) are ready as early as the aggregate ~300 GB/s allows.
            nc.sync.dma_start(out=wk_sb[:, 0, :], in_=wkT.ap()[0:P, :])
            nc.sync.dma_start(out=bk_sb[:], in_=bk.ap().rearrange("(m p) -> p m", p=P))
            xk_t = [xs.tile([P, S], BF16, tag="x", name=f"xk{dj}") for dj in range(DJ)]
            for dj in range(1, DJ):
                nc.sync.dma_start(out=wk_sb[:, dj, :], in_=wkT.ap()[dj * P:(dj + 1) * P, :])
            for dj in range(DJ):
                eng = nc.sync if dj % 2 == 0 else nc.gpsimd
                eng.dma_start(out=xk_t[dj][:, :n_kp], in_=xkT.ap()[dj * P:(dj + 1) * P, :])
            nc.sync.dma_start(out=bq_sb[:], in_=bq.ap().rearrange("(m p) -> p m", p=P))
            bv_rep = pp.tile([P, CH], F32, name="bv_rep")
            nc.gpsimd.dma_start(out=bv_rep[:], in_=bv.ap()[None, :].partition_broadcast(P))
            valid_sb = pp.tile([P, NJ], F32, name="valid_sb")
            nc.sync.dma_start(out=valid_sb[:], in_=valid.ap().rearrange("(j p) -> p j", p=P))
            valid_bf = pp.tile([P, NJ], BF16, name="valid_bf")
            nc.vector.tensor_copy(out=valid_bf[:], in_=valid_sb[:])

            for dj in range(DJ):
                nc.sync.dma_start(out=wq_sb[:, dj, :], in_=wqT.ap()[dj * P:(dj + 1) * P, :])
            xq_t = [xs.tile([P, S], BF16, tag="x", name=f"xq{dj}") for dj in range(DJ)]
            xv_t = [xs.tile([P, S], BF16, tag="x", name=f"xv{dj}") for dj in range(DJ)]
            qcnt = 0

            def xq_block(qb):
                nonlocal_marker = 0  # noqa
                for dj in range(DJ):
                    eng = nc.sync if dj % 4 != 3 else nc.gpsimd
                    eng.dma_start(
                        out=xq_t[dj][:, qb * 512:(qb + 1) * 512],
                        in_=xqT.ap()[dj * P:(dj + 1) * P, qb * 512:(qb + 1) * 512])

            xq_block(0)
            xq_block(1)
            for dj in range(DJ):
                nc.gpsimd.dma_start(out=wv_sb[:, dj, :], in_=wvT.ap()[dj * P:(dj + 1) * P, :])
            for dj in range(DJ):
                eng = nc.sync if dj % 2 == 0 else nc.gpsimd
                eng.dma_start(out=xv_t[dj][:, :n_kp], in_=xvT.ap()[dj * P:(dj + 1) * P, :])
            xq_block(2)
            xq_block(3)
            for m2 in range(MC):
                nc.sync.dma_start(out=wo_sb[:, m2, :], in_=woT.ap()[m2 * P:(m2 + 1) * P, :])

            # ---- constants + PE warmup ------------------------------------
            F32R = DT.float32r
            ones_f = pp.tile([P, DV], F32, name="ones_f")
            nc.vector.memset(ones_f[:, :], 1.0)
            ones_t = pp.tile([P, DV], F32R, name="ones_t")
            nc.vector.tensor_copy(out=ones_t[:, :], in_=ones_f[:, :])
            warm = pp.tile([P, 512], BF16, name="warm")
            nc.vector.memset(warm[:, :], 0.0)
            wout = scr.tile([P, 512], mybir.dt.float32, tag="s", name="wout")

            def warmup(n):
                # dependency-free matmuls that keep the PE's DVFS/ramp state
                # high while the input DMA stream is still landing (removing
                # these measured +25us: the ramp matters more than the duty
                # budget they consume)
                wps = pfill.tile([P, 512], mybir.dt.float32, tag="pf")
                for i in range(n):
                    nc.tensor.matmul(wps[:, :512], lhsT=warm[:, 0:P],
                                     rhs=warm[:, :512],
                                     start=(i == 0), stop=(i == n - 1))
                nc.vector.tensor_copy(out=wout[:], in_=wps[:, :512])
            # ---- projection group emitters (each -> one 1-bank pfill tile) -
            def kgroup(m, kb):
                w = min(512, n_kp - kb * 512)
                ps = pfill.tile([P, 512], mybir.dt.float32, tag="pf",
                                name=f"kg{m}{kb}")
                for dj in range(DJ):
                    nc.tensor.matmul(
                        ps[:, :w],
                        lhsT=wk_sb[:, dj, m * P:(m + 1) * P],
                        rhs=xk_t[dj][:, kb * 512:kb * 512 + w],
                        start=(dj == 0), stop=(dj == DJ - 1))
                nc.vector.tensor_scalar(
                    out=kT_sb[:, m, kb * 512:kb * 512 + w], in0=ps[:, :w],
                    scalar1=bk_sb[:, m:m + 1], scalar2=None, op0=ALU.add)

            def qgroup(m, qb):
                ps = pfill.tile([P, 512], mybir.dt.float32, tag="pf",
                                name=f"qg{m}{qb}")
                for dj in range(DJ):
                    nc.tensor.matmul(
                        ps[:, :512],
                        lhsT=wq_sb[:, dj, m * P:(m + 1) * P],
                        rhs=xq_t[dj][:, qb * 512:(qb + 1) * 512],
                        start=(dj == 0), stop=(dj == DJ - 1))
                nc.vector.tensor_scalar(
                    out=qT_sb[:, m, qb * 512:(qb + 1) * 512], in0=ps[:, :512],
                    scalar1=bq_sb[:, m:m + 1], scalar2=None, op0=ALU.add)

            def qgroup_gen(m, qb):
                ps = pfill.tile([P, 512], mybir.dt.float32, tag="pf",
                                name=f"qg{m}{qb}")
                for dj in range(DJ):
                    nc.tensor.matmul(
                        ps[:, :512],
                        lhsT=wq_sb[:, dj, m * P:(m + 1) * P],
                        rhs=xq_t[dj][:, qb * 512:(qb + 1) * 512],
                        start=(dj == 0), stop=(dj == DJ - 1))
                    if dj in (2, 5):
                        yield
                nc.vector.tensor_scalar(
                    out=qT_sb[:, m, qb * 512:(qb + 1) * 512], in0=ps[:, :512],
                    scalar1=bq_sb[:, m:m + 1], scalar2=None, op0=ALU.add)

            def vgroup(j):
                ps = pfill.tile([P, 512], mybir.dt.float32, tag="pf",
                                name=f"vg{j}")
                for dj in range(DJ):
                    nc.tensor.matmul(
                        ps[:, :CH],
                        lhsT=xv_t[dj][:, j * P:(j + 1) * P],
                        rhs=wv_sb[:, dj, :],
                        start=(dj == 0), stop=(dj == DJ - 1))
                vst = scr.tile([P, 1024], mybir.dt.float32, tag="s")
                nc.vector.tensor_tensor(out=vst[:, :CH], in0=ps[:, :CH],
                                        in1=bv_rep[:], op=ALU.add)
                nc.vector.tensor_scalar(
                    out=vaug[:, j, :, 0:DV],
                    in0=vst[:, :CH].rearrange("p (h d) -> p h d", h=HPG),
                    scalar1=valid_sb[:, j:j + 1], scalar2=None, op0=ALU.mult)
                for h in range(HPG):
                    nc.gpsimd.tensor_copy(out=vaug[:, j, h, DV:DV + 1],
                                          in_=valid_bf[:, j:j + 1])

            _stages = {}

            def opgroup(qc, n2):
                ps = pfill.tile([P, 512], mybir.dt.float32, tag="pf",
                                name=f"og{qc}{n2}")
                for m in range(MC):
                    nc.tensor.matmul(
                        ps[:, :512],
                        lhsT=ctxN[:, m, qc * P:(qc + 1) * P],
                        rhs=wo_sb[:, m, n2 * 512:(n2 + 1) * 512],
                        start=(m == 0), stop=(m == MC - 1))
                if n2 == 0:
                    _stages[qc] = op.tile([P, D], BF16, tag="o", name=f"os{qc}")
                stage = _stages[qc]
                nc.vector.tensor_copy(out=stage[:, n2 * 512:(n2 + 1) * 512],
                                      in_=ps[:, :512])
                if n2 == 1:
                    nc.sync.dma_start(out=out.ap()[qc * P:(qc + 1) * P, :],
                                      in_=stage[:])
                    del _stages[qc]

            # ---- attention: one flat software-pipelined stream --------------
            # All 8 (q-half, head) score loops run as a single continuous
            # iteration stream on the PE.  AV (attn @ V) matmuls trail the
            # score/exp stream through a pending queue that drains up to two
            # per iteration, so head boundaries cost no PE or ACT bubbles.
            # Head (0,0) runs scores-only: its AV backlog (and the whole V
            # projection, which is still waiting on the xv DMA) drains inside
            # head (0,1)'s iterations.
            ctx_of = {}
            pend = []           # (half, h, j, ex) awaiting their AV matmuls
            fillq = []          # filler generators/thunks; [0] may be started
            fill_started = [False]
            v_emitted = [0]     # vgroups emitted so far (gates h(0,0) AVs)

            import types as _types

            def fill_slot():
                # emit one filler chunk: generators resume (a few matmuls per
                # slot) so a single 8-matmul projection group never blocks the
                # exp stream for 3us straight.
                while fillq:
                    f = fillq[0]
                    if not isinstance(f, _types.GeneratorType):
                        fillq.pop(0)
                        f()
                        return True
                    try:
                        next(f)
                        fill_started[0] = True
                        return True
                    except StopIteration:
                        fillq.pop(0)
                        fill_started[0] = False
                return False

            def push_priority(f):
                # normalize-finish closures must precede any not-yet-started
                # static filler (out-proj readers of ctxN rely on program
                # order), but must not split an in-flight generator.
                idx = 1 if (fillq and fill_started[0]) else 0
                fillq.insert(idx, f)

            def emit_av(half, h, j, ex):
                key = (half, h)
                if key not in ctx_of:
                    ctx_of[key] = psc.tile([P, 1024], mybir.dt.float32,
                                           tag="ctx", name=f"ctx{half}{h}")
                ctx_ps = ctx_of[key]
                for qq in range(2):
                    nc.tensor.matmul(
                        ctx_ps[0:DV + 1, qq * 512:(qq + 1) * 512],
                        lhsT=vaug[:, j, h, :],
                        rhs=ex[:, qq * 512:(qq + 1) * 512],
                        start=(j == 0), stop=(j == NJ - 1))
                if j == NJ - 1:
                    finish_head(half, h)

            def finish_head(half, h):
                q0 = half * 1024
                m, po = h // 2, (h % 2) * 64
                ctx_ps = ctx_of.pop((half, h))
                ctxU = cu.tile([P, 1024], F32R, tag="cu", name=f"cu{half}{h}")
                nc.vector.tensor_copy(out=ctxU[0:DV + 1, :],
                                      in_=ctx_ps[0:DV + 1, :])

                def fin():
                    # rank-1 PE broadcast of the denominator row, fast
                    # custom-DVE reciprocal from PSUM, then normalize.
                    rec = scr.tile([P, 1024], mybir.dt.float32, tag="s",
                                   name=f"rc{half}{h}")
                    for qq in range(2):
                        dps = pfill.tile([P, 512], mybir.dt.float32, tag="pf",
                                         name=f"dn{half}{h}{qq}")
                        nc.tensor.matmul(
                            dps[0:DV, :512], lhsT=ones_t[64:65, :],
                            rhs=ctxU[64:65, qq * 512:(qq + 1) * 512],
                            start=True, stop=True)
                        nc.vector.reciprocal_approx_fast(
                            out=rec[0:DV, qq * 512:(qq + 1) * 512],
                            in_=dps[0:DV, :512])
                    if po == 0:
                        nc.vector.tensor_tensor(out=ctxN[0:64, m, q0:q0 + 1024],
                                                in0=ctxU[0:64, :],
                                                in1=rec[0:64, :], op=ALU.mult)
                    else:
                        tmp = scr.tile([P, 1024], BF16, tag="s",
                                       name=f"tm{half}{h}")
                        nc.vector.tensor_tensor(out=tmp[0:64, :],
                                                in0=ctxU[0:64, :],
                                                in1=rec[0:64, :], op=ALU.mult)
                        nc.gpsimd.dma_start(out=ctxN[64:128, m, q0:q0 + 1024],
                                            in_=tmp[0:64, :])
                push_priority(fin)

            def drain_av(limit, keep=1):
                n = 0
                while n < limit and len(pend) > keep:
                    half, h, j, ex = pend[0]
                    if (half, h) == (0, 0) and j >= v_emitted[0]:
                        break       # vaug[j] not emitted yet
                    pend.pop(0)
                    emit_av(half, h, j, ex)
                    n += 1

            def flat_head(half, h, fillers=(), av=True):
                for f in fillers:
                    fillq.append(f)
                q0 = half * 1024
                m, po = h // 2, (h % 2) * 64
                for j in range(NJ):
                    st = psw.tile([P, 1024], mybir.dt.float32, tag="ps",
                                  name=f"st{half}{h}{j}")
                    for qq in range(2):
                        nc.tensor.matmul(
                            st[:, qq * 512:(qq + 1) * 512],
                            lhsT=kT_sb[po:po + 64, m, j * P:(j + 1) * P],
                            rhs=qT_sb[po:po + 64, m,
                                      q0 + qq * 512:q0 + (qq + 1) * 512],
                            start=True, stop=True)
                    ex = ep.tile([P, 1024], BF16, tag="e", name=f"ex{half}{h}{j}")
                    nc.scalar.activation(out=ex[:], in_=st[:], func=AF.Exp)
                    pend.append((half, h, j, ex))
                    if av:
                        drain_av(2)
                    fill_slot()
                # drain this head's leftover fillers before moving on
                while fillq:
                    fill_slot()

            warmup(24)
            for m in range(MC):
                for kb in range(NKB):
                    kgroup(m, kb)
            qgroup(0, 0)
            qgroup(0, 1)
            flat_head(0, 0, av=False)
            for j in range(NJ):
                vgroup(j)
                v_emitted[0] = j + 1
                drain_av(1)
            flat_head(0, 1, fillers=[qgroup_gen(1, 0), qgroup_gen(1, 1)])
            flat_head(0, 2, fillers=[qgroup_gen(0, 2), qgroup_gen(0, 3)])
            flat_head(0, 3, fillers=[qgroup_gen(1, 2), qgroup_gen(1, 3)])
            flat_head(1, 1, fillers=[lambda: opgroup(0, 0), lambda: opgroup(0, 1)])
            flat_head(1, 3, fillers=[
                lambda: opgroup(1, 0), lambda: opgroup(1, 1),
                lambda: opgroup(2, 0), lambda: opgroup(2, 1)])
            flat_head(1, 0, fillers=[
                lambda: opgroup(3, 0), lambda: opgroup(3, 1),
                lambda: opgroup(4, 0), lambda: opgroup(4, 1)])
            flat_head(1, 2, fillers=[
                lambda: opgroup(5, 0), lambda: opgroup(5, 1),
                lambda: opgroup(6, 0), lambda: opgroup(6, 1),
                lambda: opgroup(7, 0), lambda: opgroup(7, 1)])
            drain_av(len(pend), keep=0)
            while fillq:
                fill_slot()

            # tail out-projection on fat [128,1024] tiles (attention PSUM is
            # free by now): half the instruction count of the 512-wide form.
            for qc in range(8, NQC):
                ps = psw.tile([P, 1024], mybir.dt.float32, tag="ps",
                              name=f"tops{qc}")
                for n2 in range(2):
                    for m in range(MC):
                        nc.tensor.matmul(
                            ps[:, n2 * 512:(n2 + 1) * 512],
                            lhsT=ctxN[:, m, qc * P:(qc + 1) * P],
                            rhs=wo_sb[:, m, n2 * 512:(n2 + 1) * 512],
                            start=(m == 0), stop=(m == MC - 1))
                stage = op.tile([P, D], BF16, tag="o", name=f"tos{qc}")
                nc.vector.tensor_copy(out=stage[:], in_=ps[:])
                eng = (nc.sync, nc.scalar, nc.gpsimd)[qc % 3]
                eng.dma_start(out=out.ap()[qc * P:(qc + 1) * P, :], in_=stage[:])

    nc.compile()
    return nc


def _ensure_axon_hooks():
    """bass_utils imports antenv.axon_hooks when tracing; this image's antenv
    lacks it. Provide it, backed by the ctypes NTFF hook when available."""
    import sys
    import types
    try:
        import antenv.axon_hooks  # noqa: F401
        return
    except ImportError:
        pass
    hook = None
    try:
        from trn_agent_boot.trn_boot import _ntff_profile_via_ctypes
        hook = _ntff_profile_via_ctypes("/opt/axon/libaxon_pjrt.so")
    except Exception:
        hook = None
    mod = types.ModuleType("antenv.axon_hooks")
    mod._hook = hook
    mod.get_axon_ntff_profile_hook = lambda: mod._hook
    mod.set_axon_ntff_profile_hook = lambda h: setattr(mod, "_hook", h)
    sys.modules["antenv.axon_hooks"] = mod


def kernel(Q, K, V, atte_mask_out, Wq, bq, Wk, bk, Wv, bv, Wo, bo):
    import jax  # noqa: F401  (must be imported first so the axon backend registers)
    from concourse.bass_utils import run_bass_kernel_spmd
    global LAST_RESULTS
    _ensure_axon_hooks()

    Q = np.asarray(Q); K = np.asarray(K); V = np.asarray(V)
    mask = np.asarray(atte_mask_out).reshape(B, S)
    Wq = np.asarray(Wq); Wk = np.asarray(Wk); Wv = np.asarray(Wv); Wo = np.asarray(Wo)
    bq = np.asarray(bq); bk = np.asarray(bk); bv = np.asarray(bv); bo = np.asarray(bo)

    keep = [np.flatnonzero(~mask[b]) for b in range(B)]
    n_kp = max(P, max(((len(ix) + P - 1) // P) * P for ix in keep))

    # per-batch packed (and bf16-rounded) tensors
    xqT, xkT, xvT, validv = [], [], [], []
    for b in range(B):
        ix = keep[b]
        xqT.append(_bf16(Q[b].T))
        kk = np.zeros((D, n_kp), np.float32)
        vv = np.zeros((D, n_kp), np.float32)
        kk[:, :len(ix)] = K[b][ix].T
        vv[:, :len(ix)] = V[b][ix].T
        xkT.append(_bf16(kk))
        xvT.append(_bf16(vv))
        va = np.zeros(n_kp, np.float32)
        va[:len(ix)] = 1.0
        validv.append(va)

    in_maps = []
    for c in range(NCORES):
        b, g = c // GROUPS, c % GROUPS
        sl = slice(g * CH, (g + 1) * CH)
        in_maps.append({
            "xqT": xqT[b], "xkT": xkT[b], "xvT": xvT[b],
            "wqT": _bf16(Wq[sl].T / SCALE),
            "wkT": _bf16(Wk[sl].T),
            "wvT": _bf16(Wv[sl].T),
            "woT": _bf16(Wo[:, sl].T),
            "bq": np.ascontiguousarray(bq[sl] / SCALE, np.float32),
            "bk": np.ascontiguousarray(bk[sl], np.float32),
            "bv": np.ascontiguousarray(bv[sl], np.float32),
            "valid": validv[b],
        })

    if n_kp not in _BUILD_CACHE:
        _BUILD_CACHE[n_kp] = _build(n_kp)
    nc = _BUILD_CACHE[n_kp]

    res = run_bass_kernel_spmd(nc, in_maps, core_ids=list(range(NCORES)))
    LAST_RESULTS = res

    full = np.zeros((B, S, D), np.float32)
    full += bo.astype(np.float32)
    for c in range(NCORES):
        full[c // GROUPS] += np.asarray(res.results[c]["out"], np.float32)
    return full
